# revision 15
# baseline (speedup 1.0000x reference)
"""Multi-head causal attention on 8 TRN2 NeuronCores.

Problem: B=4, T=2048, D=1024, H=16 heads of 64. Sharding: core c handles
batch c//2 and head-group c%2 (8 heads = 512 cols of the concat dim). Each
core computes its partial output projection o_g @ Wo_g^T; the host sums the
two partials per batch and adds the bias.

Host prep per core: x transposed per batch; Q/K projection inputs packed
fp8e4m3 in DoubleRow interleave (xdr [p, kt, n, t], wq/wk [p, kt, t, m]
with weights pre-scaled x64 so fp8 sees unit-variance values; the 64*64
factor is folded into the exp scale). x bf16 (xT) and Wv bf16 feed the V
projection at full precision. woT bf16, causal mask tri2 fp32.

Per-core kernel:
  Q/K proj: fp8 DoubleRow matmuls (contraction 256/instr) -> qT/kT bf16
  (2-pair ring). V proj: bf16. scores: per stripe j, 512-col chunks; the
  two heads' K=64 matmuls write adjacent PSUM banks of one [128,1024]
  tile on disjoint PE row groups (concurrent); causal mask = additive
  -1e6 upper-tri pre-exp; one exp per chunk covers both heads via 3D APs
  writing bf16 per-stripe P tiles (ring: stripe tags, j=0 double-buffered
  so the next pair's first exp can start while this pair's PV drains).
  PV: out[q, 0:65|65:130] = P @ V_aug (bf16), col 64 = softmax denom;
  normalize by per-partition reciprocal, DMA-transpose into oT[hd, t].
  proj: partial[t,:] = oT^T @ WoT_g, lagged 3 tiles behind pair 3's PV so
  the ~1.2us DMA transposes stay off the PE critical path; proj + pair-3
  normalize evacuations run on ScalarE (idle after the last exp).
  ~48 warmup matmuls hold the PE HAM clock-gate warm through the DMA
  phase; qkT units of pair p+1 fill pair p's ACT-bound gaps.
"""

import numpy as np
import ml_dtypes
from contextlib import ExitStack

import concourse.mybir as mybir
import concourse.tile as tile
from concourse import bacc
from concourse.bass_utils import run_bass_kernel_spmd

F32 = mybir.dt.float32
BF16 = mybir.dt.bfloat16
FP8E4 = mybir.dt.float8e4
DR = mybir.MatmulPerfMode.DoubleRow

B, T, D, H = 4, 2048, 1024, 16
HD = 64          # head dim
HG = 8           # heads per core
GW = HG * HD     # 512, group width
NT = T // 128    # 16 t-tiles
NK = D // 128    # 8 d-tiles
NKT = 4          # DoubleRow k-pair tiles (contraction 256 each)
N_CORES = 8
N_WARMUP = 48
W_SCALE = 64.0   # fp8 Wq/Wk pre-scale; folded into EXP_SCALE
EXP_SCALE = 0.125 / (W_SCALE * W_SCALE)


def _build():
    nc = bacc.Bacc("TRN2", target_bir_lowering=False, debug=False,
                   num_devices=N_CORES)
    xT_d = nc.dram_tensor("xT", [D, T], BF16, kind="ExternalInput").ap()
    xdr_d = nc.dram_tensor("xdr", [128, NKT * T * 2], FP8E4,
                           kind="ExternalInput").ap()
    wqdr_d = nc.dram_tensor("wqdr", [128, NKT * 2 * GW], FP8E4,
                            kind="ExternalInput").ap()
    wkdr_d = nc.dram_tensor("wkdr", [128, NKT * 2 * GW], FP8E4,
                            kind="ExternalInput").ap()
    wv_d = nc.dram_tensor("wv", [D, GW], BF16, kind="ExternalInput").ap()
    wo_d = nc.dram_tensor("woT", [GW, D], BF16, kind="ExternalInput").ap()
    tri_d = nc.dram_tensor("tri2", [128, 256], F32, kind="ExternalInput").ap()
    out_d = nc.dram_tensor("out", [T, D], F32, kind="ExternalOutput").ap()

    with tile.TileContext(nc) as tc, ExitStack() as ctx:
        perm = ctx.enter_context(tc.tile_pool(name="perm", bufs=1))
        psA = ctx.enter_context(tc.tile_pool(name="psA", bufs=2, space="PSUM"))
        psB = ctx.enter_context(tc.tile_pool(name="psB", bufs=2, space="PSUM"))
        ps_o = ctx.enter_context(tc.tile_pool(name="ps_o", bufs=2, space="PSUM"))

        tri2 = perm.tile([128, 256], F32, tag="tri2")
        nc.sync.dma_start(tri2[:], tri_d[:])
        tri3 = tri2.rearrange("p (h w) -> p h w", h=2)

        qT = perm.tile([128, 2, T], BF16, tag="qT")
        kT = perm.tile([128, 2, T], BF16, tag="kT")
        vsb = perm.tile([128, NT, HG * (HD + 1)], BF16, tag="vsb")
        wob = perm.tile([128, 4, D], BF16, tag="wob")
        oT = perm.tile([128, 4, T], BF16, tag="oT")
        wsrc = perm.tile([128, 384], BF16, tag="wsrc")

        nc.vector.memset(wsrc[:], 0.0)
        # ones columns for V_aug
        vcols = vsb.rearrange("p j (h c) -> p j h c", c=HD + 1)
        nc.vector.memset(vcols[:, :, :, HD:HD + 1], 1.0)

        def scores_stripe(pair, ptj, j):
            """scoresT chunks for both heads of one stripe; each 512-chunk:
            head A -> bank0 cols, head B -> bank1 cols of one psA tile so
            the two K=64 matmuls run on disjoint PE row groups; additive
            causal mask on the diagonal chunk; one fused exp per chunk."""
            m = pair % 2
            wj = T - 128 * j
            nch = (wj + 511) // 512
            for c in range(nch):
                w = min(512, wj - 512 * c)
                qa = 128 * j + 512 * c
                ps = psA.tile([128, 1024], F32, tag="psA",
                              name=f"s{pair}_{j}_{c}")
                for hh in range(2):
                    base = 64 * hh
                    nc.tensor.matmul(
                        ps[:, 512 * hh:512 * hh + w],
                        kT[base:base + 64, m, 128 * j:128 * (j + 1)],
                        qT[base:base + 64, m, qa:qa + w],
                        start=True, stop=True)
                ps3 = ps.rearrange("p (h w) -> p h w", h=2)
                if c == 0:
                    # causal: -1e6 above the diagonal of the diag block
                    nc.vector.tensor_add(ps3[:, :, 0:128], ps3[:, :, 0:128],
                                         tri3[:])
                nc.scalar.activation(
                    ptj[:, :, 512 * c:512 * c + w], ps3[:, :, :w],
                    mybir.ActivationFunctionType.Exp, scale=EXP_SCALE)

        def pv_i(pair, pts, i, smp, scalar_norm=False, after_i=None):
            """PV + normalize + DMA-transpose into oT for one q-tile"""
            po = ps_o.tile([128, 2 * (HD + 1)], F32, tag="po")
            for hh in range(2):
                h = 2 * pair + hh
                c0 = (HD + 1) * hh
                for j in range(i + 1):
                    nc.tensor.matmul(
                        po[:, c0:c0 + HD + 1],
                        pts[j][:, hh, 128 * (i - j):128 * (i - j) + 128],
                        vsb[:, j, (HD + 1) * h:(HD + 1) * (h + 1)],
                        start=(j == 0), stop=(j == i))
            recip = smp.tile([128, 2], F32, tag="recip")
            pov = po.rearrange("p (h c) -> p h c", c=HD + 1)
            nc.vector.reciprocal(recip[:], pov[:, :, HD])
            onat = smp.tile([128, 128], BF16, tag="onat")
            for hh in range(2):
                c0 = (HD + 1) * hh
                if scalar_norm:
                    nc.scalar.mul(onat[:, 64 * hh:64 * hh + 64],
                                  po[:, c0:c0 + HD], recip[:, hh:hh + 1])
                else:
                    nc.vector.tensor_scalar_mul(
                        onat[:, 64 * hh:64 * hh + 64],
                        po[:, c0:c0 + HD], recip[:, hh:hh + 1])
            nc.sync.dma_start(oT[:, pair, 128 * i:128 * (i + 1)],
                              onat[:], transpose=True)
            if after_i is not None:
                after_i(i)

        with tc.tile_pool(name="ph1", bufs=1) as ph1, \
             tc.tile_pool(name="ptp", bufs=1) as ptp, \
             tc.tile_pool(name="sm", bufs=6) as smp, \
             tc.tile_pool(name="outp", bufs=2) as outp:
            xT = ph1.tile([128, NK, T], BF16, tag="xT")
            xdr = ph1.tile([128, NKT, T, 2], FP8E4, tag="xdr")
            wqdr = ph1.tile([128, NKT, 2, GW], FP8E4, tag="wqdr")
            wkdr = ph1.tile([128, NKT, 2, GW], FP8E4, tag="wkdr")
            wvb = ph1.tile([128, NK, GW], BF16, tag="wvb")

            # PE warmup: back-to-back matmuls with no input deps keep
            # the HAM clock-gate at K=8/8 while input DMAs stream in
            for wi in range(N_WARMUP):
                pw = psB.tile([128, 512], F32, tag="psB", name=f"warm{wi}")
                nc.tensor.matmul(pw[:, 0:384], wsrc[:, 0:128], wsrc[:],
                                 start=True, stop=True)

            qs = [nc.sync, nc.scalar, nc.gpsimd]
            # Q/K DoubleRow inputs first (first compute to run)
            for kt in range(NKT):
                qs[kt % 3].dma_start(
                    xdr[:, kt, :, :].rearrange("p a b -> p (a b)"),
                    xdr_d[:, kt * T * 2:(kt + 1) * T * 2])
                qs[(kt + 1) % 3].dma_start(
                    wqdr[:, kt, :, :].rearrange("p a b -> p (a b)"),
                    wqdr_d[:, kt * 2 * GW:(kt + 1) * 2 * GW])
                qs[(kt + 2) % 3].dma_start(
                    wkdr[:, kt, :, :].rearrange("p a b -> p (a b)"),
                    wkdr_d[:, kt * 2 * GW:(kt + 1) * 2 * GW])
            # then V-path (needed from pair 0 stripe 0) and proj weights
            for k in range(NK):
                qs[k % 3].dma_start(xT[:, k, :], xT_d[128 * k:128 * (k + 1), :])
                qs[(k + 1) % 3].dma_start(wvb[:, k, :],
                                          wv_d[128 * k:128 * (k + 1), :])
            for k in range(4):
                qs[(k + 2) % 3].dma_start(wob[:, k, :],
                                          wo_d[128 * k:128 * (k + 1), :])

            # ---- Q/K projection units (8 per m-tile), fp8 DoubleRow ----
            def qkT_unit(m, u):
                c, qk = u // 2, u % 2
                wdr, dst = ((wqdr, qT), (wkdr, kT))[qk]
                ps = psB.tile([128, 512], F32, tag="psB")
                for kt in range(NKT):
                    nc.tensor.matmul(
                        ps[:], wdr[:, kt, :, 128 * m:128 * (m + 1)],
                        xdr[:, kt, 512 * c:512 * (c + 1), :]
                        .rearrange("p n t -> p t n"),
                        start=(kt == 0), stop=(kt == NKT - 1),
                        perf_mode=DR)
                nc.vector.tensor_copy(
                    dst[:, m % 2, 512 * c:512 * (c + 1)], ps[:])

            def v_jtile(j):
                ps = psB.tile([128, 512], F32, tag="psB")
                for k in range(NK):
                    nc.tensor.matmul(ps[:],
                                     xT[:, k, 128 * j:128 * (j + 1)],
                                     wvb[:, k, :],
                                     start=(k == 0), stop=(k == NK - 1))
                nc.vector.tensor_copy(vcols[:, j, :, :HD], ps[:])

            for u in range(8):
                qkT_unit(0, u)

            def proj_i(i):
                for n in range(2):
                    ost = outp.tile([128, 512], F32, tag="ost",
                                    name=f"ost{i}_{n}")
                    ps = psB.tile([128, 512], F32, tag="psB")
                    for k in range(4):
                        nc.tensor.matmul(ps[:],
                                         oT[:, k, 128 * i:128 * (i + 1)],
                                         wob[:, k, 512 * n:512 * (n + 1)],
                                         start=(k == 0), stop=(k == 3))
                    nc.scalar.copy(ost[:], ps[:])
                    nc.gpsimd.dma_start(
                        out_d[128 * i:128 * (i + 1), 512 * n:512 * (n + 1)],
                        ost[:])

            def proj_lag(i):
                if i >= 3:
                    proj_i(i - 3)

            # ---- attention head pairs ----
            for pair in range(4):
                pts = []
                for j in range(NT):
                    ptj = ptp.tile([128, 2, T - 128 * j], BF16,
                                   tag=f"pt{j}", bufs=2 if j == 0 else 1,
                                   name=f"pt{pair}_{j}")
                    pts.append(ptj)
                    scores_stripe(pair, ptj, j)
                    if pair == 0:
                        v_jtile(j)
                    if pair < 3 and j % 2 == 1:
                        qkT_unit(pair + 1, j // 2)
                    if j > 0:
                        pv_i(pair, pts, j - 1, smp,
                             scalar_norm=(pair == 3),
                             after_i=proj_lag if pair == 3 else None)
                pv_i(pair, pts, NT - 1, smp, scalar_norm=(pair == 3),
                     after_i=proj_lag if pair == 3 else None)
            for i in (NT - 3, NT - 2, NT - 1):
                proj_i(i)

    nc.compile()
    return nc


_NC_CACHE = None


def _get_nc():
    global _NC_CACHE
    if _NC_CACHE is None:
        _NC_CACHE = _build()
    return _NC_CACHE


def _prep_in_maps(x, Wq, Wk, Wv, Wo):
    bf = ml_dtypes.bfloat16
    f8 = ml_dtypes.float8_e4m3fn
    tri = np.where(np.triu(np.ones((128, 128), dtype=bool)),
                   np.float32(0.0), np.float32(-1e6))
    tri2 = np.concatenate([tri, tri], axis=1).astype(np.float32)
    in_maps = []
    for c in range(N_CORES):
        b, g = c // 2, c % 2
        hsl = slice(HG * g, HG * (g + 1))
        xb = np.ascontiguousarray(x[b].T)  # [D, T]
        # xdr[p, kt, n, t] = x[256kt+128t+p, n]
        xdr = np.ascontiguousarray(
            xb.reshape(NKT, 2, 128, T).transpose(2, 0, 3, 1)).astype(f8)
        wq = W_SCALE * Wq[hsl].transpose(1, 0, 2).reshape(D, GW)
        wk = W_SCALE * Wk[hsl].transpose(1, 0, 2).reshape(D, GW)
        # wdr[p, kt, t, m] = W[256kt+128t+p, m]
        wqdr = np.ascontiguousarray(
            wq.reshape(NKT, 2, 128, GW).transpose(2, 0, 1, 3)).astype(f8)
        wkdr = np.ascontiguousarray(
            wk.reshape(NKT, 2, 128, GW).transpose(2, 0, 1, 3)).astype(f8)
        in_maps.append({
            "xT": xb.astype(bf),
            "xdr": xdr.reshape(128, NKT * T * 2),
            "wqdr": wqdr.reshape(128, NKT * 2 * GW),
            "wkdr": wkdr.reshape(128, NKT * 2 * GW),
            "wv": np.ascontiguousarray(
                Wv[hsl].transpose(1, 0, 2).reshape(D, GW)).astype(bf),
            "woT": np.ascontiguousarray(
                Wo[:, GW * g:GW * (g + 1)].T).astype(bf),
            "tri2": tri2,
        })
    return in_maps


def kernel(x, Wq, Wk, Wv, Wo, bo, _trace=False, _tmpdir=None):
    nc = _get_nc()
    x = np.asarray(x, dtype=np.float32)
    bo = np.asarray(bo, dtype=np.float32)
    in_maps = _prep_in_maps(x, np.asarray(Wq, np.float32),
                            np.asarray(Wk, np.float32),
                            np.asarray(Wv, np.float32),
                            np.asarray(Wo, np.float32))
    res = run_bass_kernel_spmd(nc, in_maps, core_ids=list(range(N_CORES)),
                               trace=_trace, tmpdir=_tmpdir)
    out = np.empty((B, T, D), dtype=np.float32)
    for b in range(B):
        out[b] = res.results[2 * b]["out"] + res.results[2 * b + 1]["out"] + bo
    if _trace:
        return out, res
    return out


# revision 18
# speedup vs baseline: 1.0938x; 1.0938x over previous
"""Multi-head causal attention on 8 TRN2 NeuronCores.

Problem: B=4, T=2048, D=1024, H=16 heads of 64. Sharding: core c handles
batch c//2 and head-group c%2 (8 heads = 512 cols of the concat dim). Each
core computes its partial output projection o_g @ Wo_g^T; the host sums the
two partials per batch and adds the bias.

Host prep per core: x transposed per batch; Q/K projection inputs packed
fp8e4m3 in DoubleRow interleave (xdr [p, kt, n, t], wq/wk [p, kt, t, m]
with weights pre-scaled x64 so fp8 sees unit-variance values; the 64*64
factor is folded into the exp scale). x bf16 (xT) and Wv bf16 feed the V
projection at full precision. woT bf16, causal mask tri2 fp32.

Per-core kernel:
  Q/K proj: fp8 DoubleRow matmuls (contraction 256/instr) -> qT/kT bf16
  (2-pair ring). V proj: bf16. scores: per stripe j, 512-col chunks; the
  two heads' K=64 matmuls write adjacent PSUM banks of one [128,1024]
  tile on disjoint PE row groups (concurrent); causal mask = additive
  -1e6 upper-tri pre-exp; one exp per chunk covers both heads via 3D APs
  writing bf16 per-stripe P tiles (ring: stripe tags, j=0 double-buffered
  so the next pair's first exp can start while this pair's PV drains).
  PV: out[q, 0:65|65:130] = P @ V_aug (bf16), col 64 = softmax denom;
  normalize by per-partition reciprocal, DMA-transpose into oT[hd, t].
  proj: partial[t,:] = oT^T @ WoT_g, lagged 3 tiles behind pair 3's PV so
  the ~1.2us DMA transposes stay off the PE critical path; proj + pair-3
  normalize evacuations run on ScalarE (idle after the last exp).
  ~48 warmup matmuls hold the PE HAM clock-gate warm through the DMA
  phase; qkT units of pair p+1 fill pair p's ACT-bound gaps.
"""

import numpy as np
import ml_dtypes
from contextlib import ExitStack

import concourse.mybir as mybir
import concourse.tile as tile
from concourse import bacc
from concourse.bass_utils import run_bass_kernel_spmd

F32 = mybir.dt.float32
BF16 = mybir.dt.bfloat16
FP8E4 = mybir.dt.float8e4
DR = mybir.MatmulPerfMode.DoubleRow

B, T, D, H = 4, 2048, 1024, 16
HD = 64          # head dim
HG = 8           # heads per core
GW = HG * HD     # 512, group width
NT = T // 128    # 16 t-tiles
NK = D // 128    # 8 d-tiles
NKT = 4          # DoubleRow k-pair tiles (contraction 256 each)
N_CORES = 8
N_WARMUP = 48
W_SCALE = 64.0   # fp8 Wq/Wk pre-scale; folded into EXP_SCALE
EXP_SCALE = 0.125


def _build():
    nc = bacc.Bacc("TRN2", target_bir_lowering=False, debug=False,
                   num_devices=N_CORES)
    xT_d = nc.dram_tensor("xT", [D, T], BF16, kind="ExternalInput").ap()
    wq_d = nc.dram_tensor("wq", [D, GW], BF16, kind="ExternalInput").ap()
    wk_d = nc.dram_tensor("wk", [D, GW], BF16, kind="ExternalInput").ap()
    wv_d = nc.dram_tensor("wv", [D, GW], BF16, kind="ExternalInput").ap()
    wo_d = nc.dram_tensor("woT", [GW, D], BF16, kind="ExternalInput").ap()
    tri_d = nc.dram_tensor("tri2", [128, 256], F32, kind="ExternalInput").ap()
    out_d = nc.dram_tensor("out", [T, D], F32, kind="ExternalOutput").ap()

    with tile.TileContext(nc) as tc, ExitStack() as ctx:
        perm = ctx.enter_context(tc.tile_pool(name="perm", bufs=1))
        psA = ctx.enter_context(tc.tile_pool(name="psA", bufs=2, space="PSUM"))
        psB = ctx.enter_context(tc.tile_pool(name="psB", bufs=2, space="PSUM"))
        ps_o = ctx.enter_context(tc.tile_pool(name="ps_o", bufs=2, space="PSUM"))

        tri2 = perm.tile([128, 256], F32, tag="tri2")
        nc.sync.dma_start(tri2[:], tri_d[:])
        tri3 = tri2.rearrange("p (h w) -> p h w", h=2)

        qT = perm.tile([128, 2, T], BF16, tag="qT")
        kT = perm.tile([128, 2, T], BF16, tag="kT")
        vsb = perm.tile([128, NT, HG * (HD + 1)], BF16, tag="vsb")
        wob = perm.tile([128, 4, D], BF16, tag="wob")
        oT = perm.tile([128, 4, T], BF16, tag="oT")
        wsrc = perm.tile([128, 384], BF16, tag="wsrc")

        nc.vector.memset(wsrc[:], 0.0)
        # ones columns for V_aug
        vcols = vsb.rearrange("p j (h c) -> p j h c", c=HD + 1)
        nc.vector.memset(vcols[:, :, :, HD:HD + 1], 1.0)

        def scores_stripe(pair, ptj, j):
            """scoresT chunks for both heads of one stripe; each 512-chunk:
            head A -> bank0 cols, head B -> bank1 cols of one psA tile so
            the two K=64 matmuls run on disjoint PE row groups; additive
            causal mask on the diagonal chunk; one fused exp per chunk."""
            m = pair % 2
            wj = T - 128 * j
            nch = (wj + 511) // 512
            for c in range(nch):
                w = min(512, wj - 512 * c)
                qa = 128 * j + 512 * c
                ps = psA.tile([128, 1024], F32, tag="psA",
                              name=f"s{pair}_{j}_{c}")
                for hh in range(2):
                    base = 64 * hh
                    nc.tensor.matmul(
                        ps[:, 512 * hh:512 * hh + w],
                        kT[base:base + 64, m, 128 * j:128 * (j + 1)],
                        qT[base:base + 64, m, qa:qa + w],
                        start=True, stop=True)
                ps3 = ps.rearrange("p (h w) -> p h w", h=2)
                if c == 0:
                    # causal: -1e6 above the diagonal of the diag block
                    nc.vector.tensor_add(ps3[:, :, 0:128], ps3[:, :, 0:128],
                                         tri3[:])
                nc.scalar.activation(
                    ptj[:, :, 512 * c:512 * c + w], ps3[:, :, :w],
                    mybir.ActivationFunctionType.Exp, scale=EXP_SCALE)

        def pv_i(pair, pts, i, smp, scalar_norm=False, after_i=None):
            """PV + normalize + DMA-transpose into oT for one q-tile"""
            po = ps_o.tile([128, 2 * (HD + 1)], F32, tag="po")
            for hh in range(2):
                h = 2 * pair + hh
                c0 = (HD + 1) * hh
                for j in range(i + 1):
                    nc.tensor.matmul(
                        po[:, c0:c0 + HD + 1],
                        pts[j][:, hh, 128 * (i - j):128 * (i - j) + 128],
                        vsb[:, j, (HD + 1) * h:(HD + 1) * (h + 1)],
                        start=(j == 0), stop=(j == i))
            recip = smp.tile([128, 2], F32, tag="recip")
            pov = po.rearrange("p (h c) -> p h c", c=HD + 1)
            nc.vector.reciprocal(recip[:], pov[:, :, HD])
            onat = smp.tile([128, 128], BF16, tag="onat")
            for hh in range(2):
                c0 = (HD + 1) * hh
                nc.vector.tensor_scalar_mul(
                    onat[:, 64 * hh:64 * hh + 64],
                    po[:, c0:c0 + HD], recip[:, hh:hh + 1])
            nc.sync.dma_start(oT[:, pair, 128 * i:128 * (i + 1)],
                              onat[:], transpose=True)
            if after_i is not None:
                after_i(i)

        with tc.tile_pool(name="ph1", bufs=1) as ph1, \
             tc.tile_pool(name="ptp", bufs=1) as ptp, \
             tc.tile_pool(name="sm", bufs=6) as smp, \
             tc.tile_pool(name="outp", bufs=2) as outp:
            xT = ph1.tile([128, NK, T], BF16, tag="xT")
            wqb = ph1.tile([128, NK, GW], BF16, tag="wqb")
            wkb = ph1.tile([128, NK, GW], BF16, tag="wkb")
            wvb = ph1.tile([128, NK, GW], BF16, tag="wvb")

            # PE warmup: back-to-back matmuls with no input deps keep
            # the HAM clock-gate at K=8/8 while input DMAs stream in
            for wi in range(N_WARMUP):
                pw = psB.tile([128, 512], F32, tag="psB", name=f"warm{wi}")
                nc.tensor.matmul(pw[:, 0:384], wsrc[:, 0:128], wsrc[:],
                                 start=True, stop=True)

            qs = [nc.sync, nc.scalar, nc.gpsimd]
            for k in range(NK):
                qs[k % 3].dma_start(xT[:, k, :], xT_d[128 * k:128 * (k + 1), :])
                qs[(k + 1) % 3].dma_start(wqb[:, k, :],
                                          wq_d[128 * k:128 * (k + 1), :])
                qs[(k + 2) % 3].dma_start(wkb[:, k, :],
                                          wk_d[128 * k:128 * (k + 1), :])
            for k in range(NK):
                qs[k % 3].dma_start(wvb[:, k, :], wv_d[128 * k:128 * (k + 1), :])
            for k in range(4):
                qs[(k + 2) % 3].dma_start(wob[:, k, :],
                                          wo_d[128 * k:128 * (k + 1), :])

            # ---- Q/K projection units (8 per m-tile), fp8 DoubleRow ----
            def qkT_unit(m, u):
                c, qk = u // 2, u % 2
                wbt, dst = ((wqb, qT), (wkb, kT))[qk]
                ps = psB.tile([128, 512], F32, tag="psB")
                for k in range(NK):
                    nc.tensor.matmul(
                        ps[:], wbt[:, k, 128 * m:128 * (m + 1)],
                        xT[:, k, 512 * c:512 * (c + 1)],
                        start=(k == 0), stop=(k == NK - 1))
                nc.vector.tensor_copy(
                    dst[:, m % 2, 512 * c:512 * (c + 1)], ps[:])

            def v_jtile(j):
                ps = psB.tile([128, 512], F32, tag="psB")
                for k in range(NK):
                    nc.tensor.matmul(ps[:],
                                     xT[:, k, 128 * j:128 * (j + 1)],
                                     wvb[:, k, :],
                                     start=(k == 0), stop=(k == NK - 1))
                nc.vector.tensor_copy(vcols[:, j, :, :HD], ps[:])

            for u in range(8):
                qkT_unit(0, u)

            def proj_i(i):
                for n in range(2):
                    ost = outp.tile([128, 512], F32, tag="ost",
                                    name=f"ost{i}_{n}")
                    ps = psB.tile([128, 512], F32, tag="psB")
                    for k in range(4):
                        nc.tensor.matmul(ps[:],
                                         oT[:, k, 128 * i:128 * (i + 1)],
                                         wob[:, k, 512 * n:512 * (n + 1)],
                                         start=(k == 0), stop=(k == 3))
                    nc.vector.tensor_copy(ost[:], ps[:])
                    nc.gpsimd.dma_start(
                        out_d[128 * i:128 * (i + 1), 512 * n:512 * (n + 1)],
                        ost[:])

            def proj_lag(i):
                if i >= 3:
                    proj_i(i - 3)

            # ---- attention head pairs ----
            for pair in range(4):
                pts = []
                for j in range(NT):
                    ptj = ptp.tile([128, 2, T - 128 * j], FP8E4,
                                   tag=f"pt{j}", bufs=2 if j == 0 else 1,
                                   name=f"pt{pair}_{j}")
                    pts.append(ptj)
                    scores_stripe(pair, ptj, j)
                    if pair == 0:
                        v_jtile(j)
                    if pair < 3 and j % 2 == 1:
                        qkT_unit(pair + 1, j // 2)
                    if j > 0:
                        pv_i(pair, pts, j - 1, smp,
                             scalar_norm=(pair == 3),
                             after_i=proj_lag if pair == 3 else None)
                pv_i(pair, pts, NT - 1, smp, scalar_norm=(pair == 3),
                     after_i=proj_lag if pair == 3 else None)
            for i in (NT - 3, NT - 2, NT - 1):
                proj_i(i)

    nc.compile()
    return nc


_NC_CACHE = None


def _get_nc():
    global _NC_CACHE
    if _NC_CACHE is None:
        _NC_CACHE = _build()
    return _NC_CACHE


def _prep_in_maps(x, Wq, Wk, Wv, Wo):
    bf = ml_dtypes.bfloat16
    f8 = ml_dtypes.float8_e4m3fn
    tri = np.where(np.triu(np.ones((128, 128), dtype=bool)),
                   np.float32(0.0), np.float32(-1e6))
    tri2 = np.concatenate([tri, tri], axis=1).astype(np.float32)
    in_maps = []
    for c in range(N_CORES):
        b, g = c // 2, c % 2
        hsl = slice(HG * g, HG * (g + 1))
        xb = np.ascontiguousarray(x[b].T)  # [D, T]
        in_maps.append({
            "xT": xb.astype(bf),
            "wq": np.ascontiguousarray(
                Wq[hsl].transpose(1, 0, 2).reshape(D, GW)).astype(bf),
            "wk": np.ascontiguousarray(
                Wk[hsl].transpose(1, 0, 2).reshape(D, GW)).astype(bf),
            "wv": np.ascontiguousarray(
                Wv[hsl].transpose(1, 0, 2).reshape(D, GW)).astype(bf),
            "woT": np.ascontiguousarray(
                Wo[:, GW * g:GW * (g + 1)].T).astype(bf),
            "tri2": tri2,
        })
    return in_maps


def kernel(x, Wq, Wk, Wv, Wo, bo, _trace=False, _tmpdir=None):
    nc = _get_nc()
    x = np.asarray(x, dtype=np.float32)
    bo = np.asarray(bo, dtype=np.float32)
    in_maps = _prep_in_maps(x, np.asarray(Wq, np.float32),
                            np.asarray(Wk, np.float32),
                            np.asarray(Wv, np.float32),
                            np.asarray(Wo, np.float32))
    res = run_bass_kernel_spmd(nc, in_maps, core_ids=list(range(N_CORES)),
                               trace=_trace, tmpdir=_tmpdir)
    out = np.empty((B, T, D), dtype=np.float32)
    for b in range(B):
        out[b] = res.results[2 * b]["out"] + res.results[2 * b + 1]["out"] + bo
    if _trace:
        return out, res
    return out


# revision 26
# speedup vs baseline: 1.1781x; 1.0771x over previous
"""Multi-head causal attention on 8 TRN2 NeuronCores.

Problem: B=4, T=2048, D=1024, H=16 heads of 64. Sharding: core c handles
batch c//2 and head-group c%2 (8 heads = 512 cols of the concat dim). Each
core computes its partial output projection o_g @ Wo_g^T; the host sums the
two partials per batch and adds the bias.

Host prep per core: x transposed per batch; Q/K projection inputs packed
fp8e4m3 in DoubleRow interleave (xdr [p, kt, n, t], wq/wk [p, kt, t, m]
with weights pre-scaled x64 so fp8 sees unit-variance values; the 64*64
factor is folded into the exp scale). x bf16 (xT) and Wv bf16 feed the V
projection at full precision. woT bf16, causal mask tri2 fp32.

Per-core kernel:
  Q/K proj: fp8 DoubleRow matmuls (contraction 256/instr) -> qT/kT bf16
  (2-pair ring). V proj: bf16. scores: per stripe j, 512-col chunks; the
  two heads' K=64 matmuls write adjacent PSUM banks of one [128,1024]
  tile on disjoint PE row groups (concurrent); causal mask = additive
  -1e6 upper-tri pre-exp; one exp per chunk covers both heads via 3D APs
  writing bf16 per-stripe P tiles (ring: stripe tags, j=0 double-buffered
  so the next pair's first exp can start while this pair's PV drains).
  PV: out[q, 0:65|65:130] = P @ V_aug (bf16), col 64 = softmax denom;
  normalize by per-partition reciprocal, DMA-transpose into oT[hd, t].
  proj: partial[t,:] = oT^T @ WoT_g, lagged 3 tiles behind pair 3's PV so
  the ~1.2us DMA transposes stay off the PE critical path.
  ~48 warmup matmuls hold the PE HAM clock-gate warm through the DMA
  phase; qkT units of pair p+1 fill pair p's ACT-bound gaps.
"""

import numpy as np
import ml_dtypes
from contextlib import ExitStack

import concourse.mybir as mybir
import concourse.tile as tile
from concourse import bacc
from concourse.bass_utils import run_bass_kernel_spmd

F32 = mybir.dt.float32
BF16 = mybir.dt.bfloat16
FP8E4 = mybir.dt.float8e4
DR = mybir.MatmulPerfMode.DoubleRow

B, T, D, H = 4, 2048, 1024, 16
HD = 64          # head dim
HG = 8           # heads per core
GW = HG * HD     # 512, group width
NT = T // 128    # 16 t-tiles
NK = D // 128    # 8 d-tiles
NKT = 4          # DoubleRow k-pair tiles (contraction 256 each)
N_CORES = 8
N_WARMUP = 62
W_SCALE = 64.0   # fp8 Wq/Wk pre-scale; folded into EXP_SCALE
EXP_SCALE = 0.125 / (W_SCALE * W_SCALE)


def _build():
    nc = bacc.Bacc("TRN2", target_bir_lowering=False, debug=False,
                   num_devices=N_CORES)
    xT_d = nc.dram_tensor("xT", [D, T], BF16, kind="ExternalInput").ap()
    xdr_d = nc.dram_tensor("xdr", [128, NKT * T * 2], FP8E4,
                           kind="ExternalInput").ap()
    wqdr_d = nc.dram_tensor("wqdr", [128, NKT * 2 * GW], FP8E4,
                            kind="ExternalInput").ap()
    wkdr_d = nc.dram_tensor("wkdr", [128, NKT * 2 * GW], FP8E4,
                            kind="ExternalInput").ap()
    wv_d = nc.dram_tensor("wv", [D, GW], BF16, kind="ExternalInput").ap()
    wo_d = nc.dram_tensor("woT", [GW, D], BF16, kind="ExternalInput").ap()
    tri_d = nc.dram_tensor("tri2", [128, 256], BF16, kind="ExternalInput").ap()
    out_d = nc.dram_tensor("out", [T, D], F32, kind="ExternalOutput").ap()

    with tile.TileContext(nc) as tc, ExitStack() as ctx:
        perm = ctx.enter_context(tc.tile_pool(name="perm", bufs=1))
        psA = ctx.enter_context(tc.tile_pool(name="psA", bufs=2, space="PSUM"))
        psB = ctx.enter_context(tc.tile_pool(name="psB", bufs=2, space="PSUM"))
        ps_o = ctx.enter_context(tc.tile_pool(name="ps_o", bufs=2, space="PSUM"))

        tri2 = perm.tile([128, 256], BF16, tag="tri2")
        nc.sync.dma_start(tri2[:], tri_d[:])
        tri3 = tri2.rearrange("p (h w) -> p h w", h=2)

        qT = perm.tile([128, 2, T], BF16, tag="qT")
        kT = perm.tile([128, 2, T], BF16, tag="kT")
        vsb = perm.tile([128, NT, HG * (HD + 1)], BF16, tag="vsb")
        wob = perm.tile([128, 4, D], BF16, tag="wob")
        oT = perm.tile([128, 4, T], BF16, tag="oT")
        wsrc = perm.tile([128, 384], BF16, tag="wsrc")

        nc.vector.memset(wsrc[:], 0.0)
        # ones columns for V_aug
        vcols = vsb.rearrange("p j (h c) -> p j h c", c=HD + 1)
        nc.vector.memset(vcols[:, :, :, HD:HD + 1], 1.0)

        def scores_stripe(pair, ptj, j):
            """scoresT chunks for both heads of one stripe; each 512-chunk:
            head A -> bank0 cols, head B -> bank1 cols of one psA tile so
            the two K=64 matmuls run on disjoint PE row groups; additive
            causal mask on the diagonal chunk; one fused exp per chunk."""
            m = pair % 2
            wj = T - 128 * j
            nch = (wj + 511) // 512
            for c in range(nch):
                w = min(512, wj - 512 * c)
                qa = 128 * j + 512 * c
                ps = psA.tile([128, 1024], F32, tag="psA",
                              name=f"s{pair}_{j}_{c}")
                for hh in range(2):
                    base = 64 * hh
                    nc.tensor.matmul(
                        ps[:, 512 * hh:512 * hh + w],
                        kT[base:base + 64, m, 128 * j:128 * (j + 1)],
                        qT[base:base + 64, m, qa:qa + w],
                        start=True, stop=True)
                ps3 = ps.rearrange("p (h w) -> p h w", h=2)
                nc.scalar.activation(
                    ptj[:, :, 512 * c:512 * c + w], ps3[:, :, :w],
                    mybir.ActivationFunctionType.Exp, scale=EXP_SCALE)
                if c == 0:
                    # causal: zero strictly-below-diagonal P after exp; off
                    # the ACT critical path (PV consumes a stripe later)
                    nc.vector.tensor_mul(ptj[:, :, 0:128], ptj[:, :, 0:128],
                                         tri3[:])

        def pv_i(pair, pts, i, smp, scalar_norm=False, after_i=None):
            """PV + normalize + DMA-transpose into oT for one q-tile"""
            po = ps_o.tile([128, 2 * (HD + 1)], F32, tag="po")
            for hh in range(2):
                h = 2 * pair + hh
                c0 = (HD + 1) * hh
                for j in range(i + 1):
                    nc.tensor.matmul(
                        po[:, c0:c0 + HD + 1],
                        pts[j][:, hh, 128 * (i - j):128 * (i - j) + 128],
                        vsb[:, j, (HD + 1) * h:(HD + 1) * (h + 1)],
                        start=(j == 0), stop=(j == i))
            recip = smp.tile([128, 2], F32, tag="recip")
            pov = po.rearrange("p (h c) -> p h c", c=HD + 1)
            nc.vector.reciprocal(recip[:], pov[:, :, HD])
            onat = smp.tile([128, 128], BF16, tag="onat")
            for hh in range(2):
                c0 = (HD + 1) * hh
                nc.vector.tensor_scalar_mul(
                    onat[:, 64 * hh:64 * hh + 64],
                    po[:, c0:c0 + HD], recip[:, hh:hh + 1])
            nc.sync.dma_start(oT[:, pair, 128 * i:128 * (i + 1)],
                              onat[:], transpose=True)
            if after_i is not None:
                after_i(i)

        with tc.tile_pool(name="ph1", bufs=1) as ph1, \
             tc.tile_pool(name="ptp", bufs=1) as ptp, \
             tc.tile_pool(name="sm", bufs=6) as smp, \
             tc.tile_pool(name="outp", bufs=2) as outp:
            xT = ph1.tile([128, NK, T], BF16, tag="xT")
            xdr = ph1.tile([128, NKT, T, 2], FP8E4, tag="xdr")
            wqdr = ph1.tile([128, NKT, 2, GW], FP8E4, tag="wqdr")
            wkdr = ph1.tile([128, NKT, 2, GW], FP8E4, tag="wkdr")
            wvb = ph1.tile([128, NK, GW], BF16, tag="wvb")

            # PE warmup: back-to-back matmuls with no input deps keep
            # the HAM clock-gate at K=8/8 while input DMAs stream in
            for wi in range(N_WARMUP):
                pw = psB.tile([128, 512], F32, tag="psB", name=f"warm{wi}")
                nc.tensor.matmul(pw[:, 0:384], wsrc[:, 0:128], wsrc[:],
                                 start=True, stop=True)

            qs = [nc.sync, nc.scalar, nc.gpsimd]
            # Q/K DoubleRow inputs first (first compute to run)
            for kt in range(NKT):
                qs[kt % 3].dma_start(
                    xdr[:, kt, :, :].rearrange("p a b -> p (a b)"),
                    xdr_d[:, kt * T * 2:(kt + 1) * T * 2])
                qs[(kt + 1) % 3].dma_start(
                    wqdr[:, kt, :, :].rearrange("p a b -> p (a b)"),
                    wqdr_d[:, kt * 2 * GW:(kt + 1) * 2 * GW])
                qs[(kt + 2) % 3].dma_start(
                    wkdr[:, kt, :, :].rearrange("p a b -> p (a b)"),
                    wkdr_d[:, kt * 2 * GW:(kt + 1) * 2 * GW])
            # then V-path (needed from pair 0 stripe 0) and proj weights
            for k in range(NK):
                qs[k % 3].dma_start(xT[:, k, :], xT_d[128 * k:128 * (k + 1), :])
                qs[(k + 1) % 3].dma_start(wvb[:, k, :],
                                          wv_d[128 * k:128 * (k + 1), :])
            for k in range(4):
                qs[(k + 2) % 3].dma_start(wob[:, k, :],
                                          wo_d[128 * k:128 * (k + 1), :])

            # ---- Q/K projection units (8 per m-tile), fp8 DoubleRow ----
            def qkT_unit(m, u):
                c, qk = u // 2, u % 2
                wdr, dst = ((wqdr, qT), (wkdr, kT))[qk]
                ps = psB.tile([128, 512], F32, tag="psB")
                for kt in range(NKT):
                    nc.tensor.matmul(
                        ps[:], wdr[:, kt, :, 128 * m:128 * (m + 1)],
                        xdr[:, kt, 512 * c:512 * (c + 1), :]
                        .rearrange("p n t -> p t n"),
                        start=(kt == 0), stop=(kt == NKT - 1),
                        perf_mode=DR)
                nc.vector.tensor_copy(
                    dst[:, m % 2, 512 * c:512 * (c + 1)], ps[:])

            def v_jtile(j):
                ps = psB.tile([128, 512], F32, tag="psB")
                for k in range(NK):
                    nc.tensor.matmul(ps[:],
                                     xT[:, k, 128 * j:128 * (j + 1)],
                                     wvb[:, k, :],
                                     start=(k == 0), stop=(k == NK - 1))
                nc.vector.tensor_copy(vcols[:, j, :, :HD], ps[:])

            for u in range(8):
                qkT_unit(0, u)

            def proj_i(i):
                for n in range(2):
                    ost = outp.tile([128, 512], F32, tag="ost",
                                    name=f"ost{i}_{n}")
                    ps = psB.tile([128, 512], F32, tag="psB")
                    for k in range(4):
                        nc.tensor.matmul(ps[:],
                                         oT[:, k, 128 * i:128 * (i + 1)],
                                         wob[:, k, 512 * n:512 * (n + 1)],
                                         start=(k == 0), stop=(k == 3))
                    nc.vector.tensor_copy(ost[:], ps[:])
                    nc.gpsimd.dma_start(
                        out_d[128 * i:128 * (i + 1), 512 * n:512 * (n + 1)],
                        ost[:])

            def proj_lag(i):
                if i >= 3:
                    proj_i(i - 3)

            # ---- attention head pairs ----
            for pair in range(4):
                pts = []
                for j in range(NT):
                    ptj = ptp.tile([128, 2, T - 128 * j], BF16,
                                   tag=f"pt{j}", bufs=2 if j == 0 else 1,
                                   name=f"pt{pair}_{j}")
                    pts.append(ptj)
                    scores_stripe(pair, ptj, j)
                    if pair == 0:
                        v_jtile(j)
                    if pair < 3 and j % 2 == 1:
                        qkT_unit(pair + 1, j // 2)
                    if pair == 3:
                        # proj before pv so its PSUM-evacuation copies sit
                        # early in the in-order DVE queue (psB recycles
                        # without waiting behind pv's recip/mul chain)
                        proj_lag(j)
                    if j > 0:
                        pv_i(pair, pts, j - 1, smp)
                pv_i(pair, pts, NT - 1, smp)
            for i in (NT - 3, NT - 2, NT - 1):
                proj_i(i)

    nc.compile()
    return nc


_NC_CACHE = None


def _get_nc():
    global _NC_CACHE
    if _NC_CACHE is None:
        _NC_CACHE = _build()
    return _NC_CACHE


def _prep_in_maps(x, Wq, Wk, Wv, Wo):
    bf = ml_dtypes.bfloat16
    f8 = ml_dtypes.float8_e4m3fn
    tri = np.triu(np.ones((128, 128), dtype=np.float32))
    tri2 = np.concatenate([tri, tri], axis=1).astype(bf)
    in_maps = []
    for c in range(N_CORES):
        b, g = c // 2, c % 2
        hsl = slice(HG * g, HG * (g + 1))
        xb = np.ascontiguousarray(x[b].T)  # [D, T]
        # xdr[p, kt, n, t] = x[256kt+128t+p, n]
        xdr = np.ascontiguousarray(
            xb.reshape(NKT, 2, 128, T).transpose(2, 0, 3, 1)).astype(f8)
        wq = W_SCALE * Wq[hsl].transpose(1, 0, 2).reshape(D, GW)
        wk = W_SCALE * Wk[hsl].transpose(1, 0, 2).reshape(D, GW)
        # wdr[p, kt, t, m] = W[256kt+128t+p, m]
        wqdr = np.ascontiguousarray(
            wq.reshape(NKT, 2, 128, GW).transpose(2, 0, 1, 3)).astype(f8)
        wkdr = np.ascontiguousarray(
            wk.reshape(NKT, 2, 128, GW).transpose(2, 0, 1, 3)).astype(f8)
        in_maps.append({
            "xT": xb.astype(bf),
            "xdr": xdr.reshape(128, NKT * T * 2),
            "wqdr": wqdr.reshape(128, NKT * 2 * GW),
            "wkdr": wkdr.reshape(128, NKT * 2 * GW),
            "wv": np.ascontiguousarray(
                Wv[hsl].transpose(1, 0, 2).reshape(D, GW)).astype(bf),
            "woT": np.ascontiguousarray(
                Wo[:, GW * g:GW * (g + 1)].T).astype(bf),
            "tri2": tri2,
        })
    return in_maps


def kernel(x, Wq, Wk, Wv, Wo, bo, _trace=False, _tmpdir=None):
    nc = _get_nc()
    x = np.asarray(x, dtype=np.float32)
    bo = np.asarray(bo, dtype=np.float32)
    in_maps = _prep_in_maps(x, np.asarray(Wq, np.float32),
                            np.asarray(Wk, np.float32),
                            np.asarray(Wv, np.float32),
                            np.asarray(Wo, np.float32))
    res = run_bass_kernel_spmd(nc, in_maps, core_ids=list(range(N_CORES)),
                               trace=_trace, tmpdir=_tmpdir)
    out = np.empty((B, T, D), dtype=np.float32)
    for b in range(B):
        out[b] = res.results[2 * b]["out"] + res.results[2 * b + 1]["out"] + bo
    if _trace:
        return out, res
    return out
